# revision 14
# baseline (speedup 1.0000x reference)
"""Trainium2 Bass kernel for nn_CDFVarianceLoss.

Math (per sample b, per tensor z in {pred[b], target[b]}, N = 65536):
    z' = (z - min z) / (max z - min z + 1e-6)
    h_j = sum_n exp(-(z'_n - c_j)^2 / (2*sigma^2)) + 1e-6,  c_j = j/63, j < 64
    cdf = cumsum(h / sum_j h)
    loss = mean_{b,j} (cdf_pred[b,j] - cdf_target[b,j])^2

Distribution: data-parallel over the batch — 16 samples over 8 cores,
2 samples per core.  Each core returns the per-(sample, bin) squared CDF
difference [2, 64]; the host averages.

Per-core pipeline (all fp32):
  - load z natural [128, 512]; DVE min/max reduce + GPSIMD partition
    all-reduce -> per-tensor -zmin and s = 1/(zmax-zmin+eps) broadcast cols
  - one DVE tensor_scalar: z' = (z + (-zmin)) * s ; one DVE mult: z'^2
  - DMA-reshape z', z'^2 into row-major tiles [4, 8192] whose 4 rows are
    (z'_pred, z'^2_pred, z'_target, z'^2_target)
  - PE matmul with static block-diag lhsT [4,128] = [[-2c;1] | [-2c;1]]
    computes q[j,n] = z'^2 - 2*c_j*z' for both tensors at once -> PSUM
  - ACT: exp(-alpha*q - alpha*c_j^2) with static per-partition bias and
    accum_out -> per-bin partial sums (the only O(N*BINS) exp pass)
  - DVE reduce -> h; +eps; GPSIMD segmented all-reduce -> 1/sum; normalize
  - PE matmul with static [128,64] cumsum-difference matrix -> cdf diff
  - DVE square -> DMA out
"""

import numpy as np

B = 16
N = 65536
BINS = 64
SIGMA = 0.05
EPS = 1e-6
ALPHA = 0.5 / SIGMA**2  # 200.0
NCORES = 8
SPC = B // NCORES  # samples per core
P = 128
F = N // P  # 512 natural free dim
CHUNK = 8192  # row-layout chunk (elements per rhs row tile)
NCHUNK = N // CHUNK  # 8
MMN = 512  # matmul moving free dim
ACTN = 2048  # ACT block: 4 matmuls per activation op

_CACHE = {}


def _build_nc():
    import concourse.bass as bass
    import concourse.bacc as bacc
    import concourse.tile as tile
    from concourse import mybir
    from contextlib import ExitStack

    f32 = mybir.dt.float32
    AX = mybir.AxisListType
    OP = mybir.AluOpType
    ACTF = mybir.ActivationFunctionType

    nc = bacc.Bacc()
    pred_d = nc.declare_dram_parameter("pred", [SPC, N], f32, isOutput=False)
    targ_d = nc.declare_dram_parameter("target", [SPC, N], f32, isOutput=False)
    out_d = nc.declare_dram_parameter("out_sq", [SPC, BINS], f32, isOutput=True)

    c = np.linspace(0.0, 1.0, BINS, dtype=np.float32)
    lhsT_main_np = np.zeros((4, P), np.float32)
    lhsT_main_np[0, :BINS] = -2.0 * c
    lhsT_main_np[1, :BINS] = 1.0
    lhsT_main_np[2, BINS:] = -2.0 * c
    lhsT_main_np[3, BINS:] = 1.0
    bias_np = np.concatenate([-ALPHA * c * c, -ALPHA * c * c]).reshape(P, 1)
    bias_np = bias_np.astype(np.float32)
    # cumsum-and-subtract: out[m] = sum_{k<=m} hn_pred[k] - sum_{k<=m} hn_targ[k]
    lhsT_tail_np = np.zeros((P, BINS), np.float32)
    for m in range(BINS):
        lhsT_tail_np[: m + 1, m] = 1.0
        lhsT_tail_np[BINS : BINS + m + 1, m] = -1.0

    lhsT_main_d = nc.inline_tensor(lhsT_main_np, name="lhsT_main")
    bias_d = nc.inline_tensor(bias_np, name="bias_col")
    lhsT_tail_d = nc.inline_tensor(lhsT_tail_np, name="lhsT_tail")

    with tile.TileContext(nc) as tc, ExitStack() as ctx:
        singles = ctx.enter_context(tc.tile_pool(name="singles", bufs=1))
        nat = ctx.enter_context(tc.tile_pool(name="nat", bufs=2))
        norm = ctx.enter_context(tc.tile_pool(name="norm", bufs=2))
        small = ctx.enter_context(tc.tile_pool(name="small", bufs=2))
        rows = ctx.enter_context(tc.tile_pool(name="rows", bufs=2))
        scr = ctx.enter_context(tc.tile_pool(name="scr", bufs=2))
        hp = ctx.enter_context(tc.tile_pool(name="hp", bufs=2))
        ps_pool = ctx.enter_context(tc.tile_pool(name="ps", bufs=2, space="PSUM"))

        lhsT_main_sb = singles.tile([4, P], f32)
        nc.sync.dma_start(out=lhsT_main_sb, in_=lhsT_main_d[:, :])
        bias_sb = singles.tile([P, 1], f32)
        nc.sync.dma_start(out=bias_sb, in_=bias_d[:, :])
        lhsT_tail_sb = singles.tile([P, BINS], f32)
        nc.sync.dma_start(out=lhsT_tail_sb, in_=lhsT_tail_d[:, :])

        for p in range(SPC):
            zA = nat.tile([P, F], f32, tag="zA")
            nc.sync.dma_start(out=zA, in_=pred_d[p, :].rearrange("(p f) -> p f", p=P))
            zB = nat.tile([P, F], f32, tag="zB")
            nc.sync.dma_start(out=zB, in_=targ_d[p, :].rearrange("(p f) -> p f", p=P))

            def norm_one(z, tag):
                # per-partition min/max over the free dim
                mm = small.tile([P, 2], f32, tag=f"mm{tag}")
                nc.vector.tensor_reduce(out=mm[:, 0:1], in_=z, axis=AX.X, op=OP.min)
                nc.vector.tensor_reduce(out=mm[:, 1:2], in_=z, axis=AX.X, op=OP.max)
                # bounce the 256 partial stats through DRAM, reload them
                # broadcast to every partition, and let each partition
                # finish the reduction locally (no cross-partition ops).
                bc_d = nc.dram_tensor(f"bc_{p}_{tag}", [P * 2], f32)
                nc.sync.dma_start(out=bc_d[:], in_=mm)
                base = bc_d[:]
                bcast = bass.AP(
                    tensor=base.tensor,
                    offset=base.offset,
                    ap=[[0, P]] + [list(d) for d in base.ap],
                )
                mmb = small.tile([P, P, 2], f32, tag=f"mmb{tag}")
                nc.sync.dma_start(out=mmb, in_=bcast)
                nzmin = small.tile([P, 1], f32, tag=f"nzmin{tag}")
                nc.vector.tensor_reduce(
                    out=nzmin, in_=mmb[:, :, 0:1], axis=AX.XY, op=OP.min,
                )
                nc.vector.tensor_scalar_mul(nzmin, nzmin, -1.0)
                zmax = small.tile([P, 1], f32, tag=f"zmax{tag}")
                nc.vector.tensor_reduce(
                    out=zmax, in_=mmb[:, :, 1:2], axis=AX.XY, op=OP.max
                )
                s = small.tile([P, 1], f32, tag=f"s{tag}")
                nc.vector.tensor_scalar(s, zmax, nzmin, EPS, OP.add, OP.add)
                nc.vector.reciprocal(s, s)
                zp = norm.tile([P, F], f32, tag=f"zp{tag}")
                nc.vector.tensor_scalar(zp, z, nzmin, s, OP.add, OP.mult)
                zp2 = norm.tile([P, F], f32, tag=f"zp2{tag}")
                nc.vector.tensor_mul(zp2, zp, zp)
                return zp, zp2

            zpA, zp2A = norm_one(zA, "A")
            zpB, zp2B = norm_one(zB, "B")

            hparts = hp.tile([P, NCHUNK * (CHUNK // ACTN)], f32, tag="hparts")
            pp = CHUNK // F  # natural partitions per chunk (16)
            for ch in range(NCHUNK):
                rt = rows.tile([4, CHUNK], f32, tag="rt")
                sl = slice(ch * pp, (ch + 1) * pp)
                nc.sync.dma_start(out=rt[0:1, :], in_=zpA[sl, :])
                nc.sync.dma_start(out=rt[1:2, :], in_=zp2A[sl, :])
                nc.sync.dma_start(out=rt[2:3, :], in_=zpB[sl, :])
                nc.sync.dma_start(out=rt[3:4, :], in_=zp2B[sl, :])
                for blk in range(CHUNK // ACTN):
                    ps = ps_pool.tile([P, ACTN], f32, tag="ps")
                    for k in range(ACTN // MMN):
                        col = blk * ACTN + k * MMN
                        nc.tensor.matmul(
                            ps[:, k * MMN : (k + 1) * MMN],
                            lhsT_main_sb[:, :],
                            rt[:, col : col + MMN],
                            start=True,
                            stop=True,
                        )
                    sc = scr.tile([P, ACTN], f32, tag="sc")
                    icol = ch * (CHUNK // ACTN) + blk
                    nc.scalar.activation(
                        out=sc,
                        in_=ps[:, :],
                        func=ACTF.Exp,
                        bias=bias_sb[:, 0:1],
                        scale=-ALPHA,
                        accum_out=hparts[:, icol : icol + 1],
                    )

            hcol = small.tile([P, 1], f32, tag="hcol")
            nc.vector.tensor_reduce(out=hcol, in_=hparts, axis=AX.X, op=OP.add)
            heps = small.tile([P, 1], f32, tag="heps")
            nc.vector.tensor_scalar_add(heps, hcol, EPS)
            # segmented sums over bins: rows 0:64 (pred), 64:128 (target).
            # Same DRAM-bounce-broadcast trick as in norm_one.
            sv_d = nc.dram_tensor(f"sv_{p}", [P], f32)
            nc.sync.dma_start(out=sv_d[:], in_=heps)
            base = sv_d[:]
            bcast = bass.AP(
                tensor=base.tensor,
                offset=base.offset,
                ap=[[0, P]] + [list(d) for d in base.ap],
            )
            hb = small.tile([P, P], f32, tag="hb")
            nc.sync.dma_start(out=hb, in_=bcast)
            sinv = small.tile([P, 2], f32, tag="sinv")
            nc.vector.tensor_reduce(
                out=sinv[:, 0:1], in_=hb[:, 0:BINS], axis=AX.X, op=OP.add
            )
            nc.vector.tensor_reduce(
                out=sinv[:, 1:2], in_=hb[:, BINS:P], axis=AX.X, op=OP.add
            )
            nc.vector.reciprocal(sinv, sinv)
            hn = small.tile([P, 1], f32, tag="hn")
            nc.vector.tensor_mul(hn[0:BINS, :], heps[0:BINS, :], sinv[0:BINS, 0:1])
            nc.vector.tensor_mul(hn[BINS:P, :], heps[BINS:P, :], sinv[BINS:P, 1:2])
            pst = ps_pool.tile([BINS, 1], f32, tag="ps")
            nc.tensor.matmul(pst, lhsT_tail_sb[:, :], hn, start=True, stop=True)
            sq = small.tile([BINS, 1], f32, tag="sq")
            nc.scalar.square(sq, pst)
            nc.gpsimd.dma_start(out=out_d[p, :], in_=sq[:, 0:1])

    nc.compile()
    return nc


def kernel(pred: np.ndarray, target: np.ndarray) -> np.ndarray:
    from concourse.bass_utils import run_bass_kernel_spmd

    if "nc" not in _CACHE:
        _CACHE["nc"] = _build_nc()
    nc = _CACHE["nc"]

    pred = np.ascontiguousarray(np.asarray(pred, np.float32).reshape(B, N))
    target = np.ascontiguousarray(np.asarray(target, np.float32).reshape(B, N))
    in_maps = [
        {
            "pred": pred[i * SPC : (i + 1) * SPC],
            "target": target[i * SPC : (i + 1) * SPC],
        }
        for i in range(NCORES)
    ]
    res = run_bass_kernel_spmd(nc, in_maps, list(range(NCORES)))
    sq = np.concatenate([r["out_sq"] for r in res.results], axis=0)  # [16, 64]
    return np.float32(np.mean(sq, dtype=np.float64))


# revision 16
# speedup vs baseline: 2.2809x; 2.2809x over previous
"""Trainium2 Bass kernel for nn_CDFVarianceLoss.

Math (per sample b, per tensor z in {pred[b], target[b]}, N = 65536):
    z' = (z - min z) / (max z - min z + 1e-6)
    h_j = sum_n exp(-(z'_n - c_j)^2 / (2*sigma^2)) + 1e-6,  c_j = j/63, j < 64
    cdf = cumsum(h / sum_j h)
    loss = mean_{b,j} (cdf_pred[b,j] - cdf_target[b,j])^2

Distribution: data-parallel over the batch — 16 samples over 8 cores,
2 samples per core.  Each core returns the per-(sample, bin) squared CDF
difference [2, 64]; the host averages.

Per-core pipeline:
  - load z natural [128, 512] fp32; DVE min/max reduce; DRAM bounce-
    broadcast of the 128 partial stats so every partition finishes the
    global min/max locally -> -zmin, s = 1/(zmax-zmin+eps) columns
  - DVE: z' = (z + (-zmin)) * s (fp32); z'^2 (fp32); then bf16 hi/lo
    splits z' = zhi+zlo, z'^2 = z2hi+z2lo (exact to ~2^-17 — needed
    because the exponent is amplified by alpha=200 and fp32 matmuls
    stream 4x slower than bf16 on the PE)
  - DMA-reshape the bf16 splits into row tiles [10, CHUNK] whose rows are
    (zhi, zhi, zlo, z2hi, z2lo) x {pred, target}
  - PE bf16 matmul, static block-diag lhsT [10,128] with columns
    [m_hi, m_lo, m_hi, 1, 1] (m = -2c split hi/lo):
    q[j,n] = z'^2 - 2*c_j*z' accumulated exactly in fp32 PSUM
  - ACT: exp(-alpha*q - alpha*c_j^2) with static per-partition bias and
    accum_out -> per-bin partial sums (the only O(N*BINS) pass)
  - DVE reduce -> h; +eps; DRAM bounce-broadcast segmented sums -> 1/sum
  - PE matmul with static [128,64] cumsum-difference matrix -> cdf diff
  - ACT square -> DMA out
"""

import numpy as np

B = 16
N = 65536
BINS = 64
SIGMA = 0.05
EPS = 1e-6
ALPHA = 0.5 / SIGMA**2  # 200.0
NCORES = 8
SPC = B // NCORES  # samples per core
P = 128
F = N // P  # 512 natural free dim
CHUNK = 16384  # row-layout chunk (elements per rhs row tile)
NCHUNK = N // CHUNK  # 4
MMN = 512  # matmul moving free dim (one PSUM bank of fp32 output)
ACTN = 2048  # ACT block: 4 matmuls per activation op
K = 10  # rhs rows: 5 per tensor x 2 tensors

_CACHE = {}


def _np_bf16_split(x):
    import ml_dtypes

    hi = x.astype(ml_dtypes.bfloat16).astype(np.float32)
    lo = (x - hi).astype(ml_dtypes.bfloat16).astype(np.float32)
    return hi, lo


def _build_nc():
    import concourse.bass as bass
    import concourse.bacc as bacc
    import concourse.tile as tile
    from concourse import mybir
    from contextlib import ExitStack

    f32 = mybir.dt.float32
    bf16 = mybir.dt.bfloat16
    AX = mybir.AxisListType
    OP = mybir.AluOpType
    ACTF = mybir.ActivationFunctionType

    nc = bacc.Bacc()
    pred_d = nc.declare_dram_parameter("pred", [SPC, N], f32, isOutput=False)
    targ_d = nc.declare_dram_parameter("target", [SPC, N], f32, isOutput=False)
    out_d = nc.declare_dram_parameter("out_sq", [SPC, BINS], f32, isOutput=True)

    c = np.linspace(0.0, 1.0, BINS, dtype=np.float32)
    m_hi, m_lo = _np_bf16_split(-2.0 * c)
    coeffs = np.stack([m_hi, m_lo, m_hi, np.ones(BINS, np.float32),
                       np.ones(BINS, np.float32)])  # [5, 64]
    lhsT_main_np = np.zeros((K, P), np.float32)
    lhsT_main_np[0:5, :BINS] = coeffs
    lhsT_main_np[5:10, BINS:] = coeffs
    import ml_dtypes

    lhsT_main_np = lhsT_main_np.astype(ml_dtypes.bfloat16)
    bias_np = np.concatenate([-ALPHA * c * c, -ALPHA * c * c]).reshape(P, 1)
    bias_np = bias_np.astype(np.float32)
    # cumsum-and-subtract: out[m] = sum_{k<=m} hn_pred[k] - sum_{k<=m} hn_targ[k]
    lhsT_tail_np = np.zeros((P, BINS), np.float32)
    for mcol in range(BINS):
        lhsT_tail_np[: mcol + 1, mcol] = 1.0
        lhsT_tail_np[BINS : BINS + mcol + 1, mcol] = -1.0

    lhsT_main_d = nc.inline_tensor(lhsT_main_np, name="lhsT_main")
    bias_d = nc.inline_tensor(bias_np, name="bias_col")
    lhsT_tail_d = nc.inline_tensor(lhsT_tail_np, name="lhsT_tail")

    with tile.TileContext(nc) as tc, ExitStack() as ctx:
        singles = ctx.enter_context(tc.tile_pool(name="singles", bufs=1))
        nat = ctx.enter_context(tc.tile_pool(name="nat", bufs=2))
        norm = ctx.enter_context(tc.tile_pool(name="norm", bufs=2))
        small = ctx.enter_context(tc.tile_pool(name="small", bufs=2))
        rows = ctx.enter_context(tc.tile_pool(name="rows", bufs=2))
        scr = ctx.enter_context(tc.tile_pool(name="scr", bufs=2))
        hp = ctx.enter_context(tc.tile_pool(name="hp", bufs=2))
        ps_pool = ctx.enter_context(tc.tile_pool(name="ps", bufs=2, space="PSUM"))

        # round-robin DMA issuing engines (PE and ACT stay DMA-free: they
        # are the busy engines; Sync/GpSimd sequencers are near-idle)
        dma_engines = [nc.sync, nc.gpsimd]
        dma_i = [0]

        def dma(out, in_):
            eng = dma_engines[dma_i[0] % len(dma_engines)]
            dma_i[0] += 1
            eng.dma_start(out=out, in_=in_)

        lhsT_main_sb = singles.tile([K, P], bf16)
        dma(lhsT_main_sb, lhsT_main_d[:, :])
        bias_sb = singles.tile([P, 1], f32)
        dma(bias_sb, bias_d[:, :])
        lhsT_tail_sb = singles.tile([P, BINS], f32)
        dma(lhsT_tail_sb, lhsT_tail_d[:, :])

        for p in range(SPC):
            zA = nat.tile([P, F], f32, tag="zA")
            dma(zA, pred_d[p, :].rearrange("(p f) -> p f", p=P))
            zB = nat.tile([P, F], f32, tag="zB")
            dma(zB, targ_d[p, :].rearrange("(p f) -> p f", p=P))

            def norm_one(z, tag):
                # per-partition min/max over the free dim
                mm = small.tile([P, 2], f32, tag=f"mm{tag}")
                nc.vector.tensor_reduce(out=mm[:, 0:1], in_=z, axis=AX.X, op=OP.min)
                nc.vector.tensor_reduce(out=mm[:, 1:2], in_=z, axis=AX.X, op=OP.max)
                # bounce the 256 partial stats through DRAM, reload them
                # broadcast to every partition, and let each partition
                # finish the reduction locally (no cross-partition ops).
                bc_d = nc.dram_tensor(f"bc_{p}_{tag}", [P * 2], f32)
                dma(bc_d[:], mm)
                base = bc_d[:]
                bcast = bass.AP(
                    tensor=base.tensor,
                    offset=base.offset,
                    ap=[[0, P]] + [list(d) for d in base.ap],
                )
                mmb = small.tile([P, P, 2], f32, tag=f"mmb{tag}")
                dma(mmb, bcast)
                nzmin = small.tile([P, 1], f32, tag=f"nzmin{tag}")
                nc.vector.tensor_reduce(
                    out=nzmin, in_=mmb[:, :, 0:1], axis=AX.XY, op=OP.min,
                )
                nc.vector.tensor_scalar_mul(nzmin, nzmin, -1.0)
                zmax = small.tile([P, 1], f32, tag=f"zmax{tag}")
                nc.vector.tensor_reduce(
                    out=zmax, in_=mmb[:, :, 1:2], axis=AX.XY, op=OP.max
                )
                s = small.tile([P, 1], f32, tag=f"s{tag}")
                nc.vector.tensor_scalar(s, zmax, nzmin, EPS, OP.add, OP.add)
                nc.vector.reciprocal(s, s)
                zp = norm.tile([P, F], f32, tag=f"zp{tag}")
                nc.vector.tensor_scalar(zp, z, nzmin, s, OP.add, OP.mult)
                zp2 = norm.tile([P, F], f32, tag=f"zp2{tag}")
                nc.vector.tensor_mul(zp2, zp, zp)
                # bf16 hi/lo splits (exact to ~2^-17 combined)
                zhi = norm.tile([P, F], bf16, tag=f"zhi{tag}")
                nc.vector.tensor_copy(zhi, zp)
                zlo = norm.tile([P, F], bf16, tag=f"zlo{tag}")
                nc.vector.tensor_sub(zlo, zp, zhi)
                z2hi = norm.tile([P, F], bf16, tag=f"z2hi{tag}")
                nc.vector.tensor_copy(z2hi, zp2)
                z2lo = norm.tile([P, F], bf16, tag=f"z2lo{tag}")
                nc.vector.tensor_sub(z2lo, zp2, z2hi)
                return zhi, zlo, z2hi, z2lo

            rowsA = norm_one(zA, "A")
            rowsB = norm_one(zB, "B")
            # rhs row order must match lhsT_main rows
            srcs = [rowsA[0], rowsA[0], rowsA[1], rowsA[2], rowsA[3],
                    rowsB[0], rowsB[0], rowsB[1], rowsB[2], rowsB[3]]

            hparts = hp.tile([P, NCHUNK * (CHUNK // ACTN)], f32, tag="hparts")
            pp = CHUNK // F  # natural partitions per chunk (32)
            for ch in range(NCHUNK):
                rt = rows.tile([K, CHUNK], bf16, tag="rt")
                sl = slice(ch * pp, (ch + 1) * pp)
                for r, src in enumerate(srcs):
                    dma(rt[r : r + 1, :], src[sl, :])
                for blk in range(CHUNK // ACTN):
                    ps = ps_pool.tile([P, ACTN], f32, tag="ps")
                    for k in range(ACTN // MMN):
                        col = blk * ACTN + k * MMN
                        nc.tensor.matmul(
                            ps[:, k * MMN : (k + 1) * MMN],
                            lhsT_main_sb[:, :],
                            rt[:, col : col + MMN],
                            start=True,
                            stop=True,
                        )
                    sc = scr.tile([P, ACTN], f32, tag="sc")
                    icol = ch * (CHUNK // ACTN) + blk
                    nc.scalar.activation(
                        out=sc,
                        in_=ps[:, :],
                        func=ACTF.Exp,
                        bias=bias_sb[:, 0:1],
                        scale=-ALPHA,
                        accum_out=hparts[:, icol : icol + 1],
                    )

            hcol = small.tile([P, 1], f32, tag="hcol")
            nc.vector.tensor_reduce(out=hcol, in_=hparts, axis=AX.X, op=OP.add)
            heps = small.tile([P, 1], f32, tag="heps")
            nc.vector.tensor_scalar_add(heps, hcol, EPS)
            # segmented sums over bins: rows 0:64 (pred), 64:128 (target).
            # Same DRAM-bounce-broadcast trick as in norm_one.
            sv_d = nc.dram_tensor(f"sv_{p}", [P], f32)
            dma(sv_d[:], heps)
            base = sv_d[:]
            bcast = bass.AP(
                tensor=base.tensor,
                offset=base.offset,
                ap=[[0, P]] + [list(d) for d in base.ap],
            )
            hb = small.tile([P, P], f32, tag="hb")
            dma(hb, bcast)
            sinv = small.tile([P, 2], f32, tag="sinv")
            nc.vector.tensor_reduce(
                out=sinv[:, 0:1], in_=hb[:, 0:BINS], axis=AX.X, op=OP.add
            )
            nc.vector.tensor_reduce(
                out=sinv[:, 1:2], in_=hb[:, BINS:P], axis=AX.X, op=OP.add
            )
            nc.vector.reciprocal(sinv, sinv)
            hn = small.tile([P, 1], f32, tag="hn")
            nc.vector.tensor_mul(hn[0:BINS, :], heps[0:BINS, :], sinv[0:BINS, 0:1])
            nc.vector.tensor_mul(hn[BINS:P, :], heps[BINS:P, :], sinv[BINS:P, 1:2])
            pst = ps_pool.tile([BINS, 1], f32, tag="ps")
            nc.tensor.matmul(pst, lhsT_tail_sb[:, :], hn, start=True, stop=True)
            sq = small.tile([BINS, 1], f32, tag="sq")
            nc.scalar.square(sq, pst)
            dma(out_d[p, :], sq[:, 0:1])

    nc.compile()
    return nc


def kernel(pred: np.ndarray, target: np.ndarray) -> np.ndarray:
    from concourse.bass_utils import run_bass_kernel_spmd

    if "nc" not in _CACHE:
        _CACHE["nc"] = _build_nc()
    nc = _CACHE["nc"]

    pred = np.ascontiguousarray(np.asarray(pred, np.float32).reshape(B, N))
    target = np.ascontiguousarray(np.asarray(target, np.float32).reshape(B, N))
    in_maps = [
        {
            "pred": pred[i * SPC : (i + 1) * SPC],
            "target": target[i * SPC : (i + 1) * SPC],
        }
        for i in range(NCORES)
    ]
    res = run_bass_kernel_spmd(nc, in_maps, list(range(NCORES)))
    sq = np.concatenate([r["out_sq"] for r in res.results], axis=0)  # [16, 64]
    return np.float32(np.mean(sq, dtype=np.float64))


# revision 23
# speedup vs baseline: 2.7945x; 1.2252x over previous
"""Trainium2 Bass kernel for nn_CDFVarianceLoss.

Math (per sample b, per tensor z in {pred[b], target[b]}, N = 65536):
    z' = (z - min z) / (max z - min z + 1e-6)
    h_j = sum_n exp(-(z'_n - c_j)^2 / (2*sigma^2)) + 1e-6,  c_j = j/63, j < 64
    cdf = cumsum(h / sum_j h)
    loss = mean_{b,j} (cdf_pred[b,j] - cdf_target[b,j])^2

Distribution: data-parallel over the batch — 16 samples over 8 cores,
2 samples per core.  Each core returns the per-(sample, bin) squared CDF
difference [2, 64]; the host averages.

Per-core pipeline:
  - load z natural [128, 512] fp32; DVE per-partition min/max; the
    128-way cross-partition reduction is finished via PE transpose
    (stats -> [2,128] PSUM) + tiny DVE reduces, and the resulting
    (-zmin, 1/(zmax-zmin+eps)) scalars are broadcast back to all 128
    partitions with a ones-column matmul — no DRAM round trips.
  - DVE: z' = (z + (-zmin)) * s (fp32); z'^2 (fp32); then bf16 hi/lo
    splits z' = zhi+zlo, z'^2 = z2hi+z2lo (combined exact to ~2^-17 —
    needed because the exponent is amplified by alpha=200, and bf16
    matmuls stream 4x faster than fp32 on the PE)
  - DMA-reshape the bf16 splits into row tiles [10, CHUNK] whose rows are
    (zhi, zhi, zlo, z2hi, z2lo) x {pred, target}
  - PE bf16 matmul, static block-diag lhsT [10,128] with columns
    [m_hi, m_lo, m_hi, 1, 1] (m = -2c split hi/lo):
    q[j,n] = z'^2 - 2*c_j*z' accumulated exactly in fp32 PSUM
  - ACT: exp(-alpha*q - alpha*c_j^2) with static per-partition bias and
    accum_out -> per-bin partial sums (the only O(N*BINS) pass)
  - DVE reduce -> h; +eps; segmented bin-sums + reciprocal broadcast via
    two tiny matmuls with static 0/1 block matrices
  - PE matmul with static [128,64] cumsum-difference matrix -> cdf diff
  - ACT square -> DMA out
"""

import numpy as np

B = 16
N = 65536
BINS = 64
SIGMA = 0.05
EPS = 1e-6
ALPHA = 0.5 / SIGMA**2  # 200.0
NCORES = 8
SPC = B // NCORES  # samples per core
P = 128
F = N // P  # 512 natural free dim
CHUNK = 16384  # row-layout chunk (elements per rhs row tile)
NCHUNK = N // CHUNK  # 4
MMN = 512  # matmul moving free dim (one PSUM bank of fp32 output)
ACTB = 4  # matmuls per ACT block (PSUM tile = 4 banks)
K = 10  # rhs rows: 5 per tensor x 2 tensors

_CACHE = {}


def _np_bf16_split(x):
    import ml_dtypes

    hi = x.astype(ml_dtypes.bfloat16).astype(np.float32)
    lo = (x - hi).astype(ml_dtypes.bfloat16).astype(np.float32)
    return hi, lo


def _build_nc():
    import concourse.bass as bass
    import concourse.bacc as bacc
    import concourse.tile as tile
    import ml_dtypes
    from concourse import mybir
    from contextlib import ExitStack

    f32 = mybir.dt.float32
    bf16 = mybir.dt.bfloat16
    AX = mybir.AxisListType
    OP = mybir.AluOpType
    ACTF = mybir.ActivationFunctionType

    nc = bacc.Bacc()
    pred_d = nc.declare_dram_parameter("pred", [SPC, N], f32, isOutput=False)
    targ_d = nc.declare_dram_parameter("target", [SPC, N], f32, isOutput=False)
    out_d = nc.declare_dram_parameter("out_sq", [SPC, BINS], f32, isOutput=True)

    c = np.linspace(0.0, 1.0, BINS, dtype=np.float32)
    m_hi, m_lo = _np_bf16_split(-2.0 * c)
    coeffs = np.stack([m_hi, m_lo, m_hi, np.ones(BINS, np.float32),
                       np.ones(BINS, np.float32)])  # [5, 64]
    lhsT_main_np = np.zeros((K, P), np.float32)
    lhsT_main_np[0:5, :BINS] = coeffs
    lhsT_main_np[5:10, BINS:] = coeffs
    lhsT_main_np = lhsT_main_np.astype(ml_dtypes.bfloat16)
    bias_np = np.concatenate([-ALPHA * c * c, -ALPHA * c * c]).reshape(P, 1)
    bias_np = bias_np.astype(np.float32)
    # cumsum-and-subtract: out[m] = sum_{k<=m} hn_pred[k] - sum_{k<=m} hn_targ[k]
    lhsT_tail_np = np.zeros((P, BINS), np.float32)
    for mcol in range(BINS):
        lhsT_tail_np[: mcol + 1, mcol] = 1.0
        lhsT_tail_np[BINS : BINS + mcol + 1, mcol] = -1.0
    # segmented-sum / segmented-broadcast 0/1 blocks
    blk_np = np.zeros((P, 2), np.float32)
    blk_np[:BINS, 0] = 1.0
    blk_np[BINS:, 1] = 1.0
    ones_row_np = np.ones((1, P), np.float32)
    ident_np = np.eye(P, dtype=np.float32)

    lhsT_main_d = nc.inline_tensor(lhsT_main_np, name="lhsT_main")
    bias_d = nc.inline_tensor(bias_np, name="bias_col")
    lhsT_tail_d = nc.inline_tensor(lhsT_tail_np, name="lhsT_tail")
    blk_d = nc.inline_tensor(blk_np, name="blk")
    blkT_d = nc.inline_tensor(np.ascontiguousarray(blk_np.T), name="blkT")
    ones_d = nc.inline_tensor(ones_row_np, name="ones_row")
    ident_d = nc.inline_tensor(ident_np, name="ident")

    with tile.TileContext(nc) as tc, ExitStack() as ctx:
        singles = ctx.enter_context(tc.tile_pool(name="singles", bufs=1))
        nat = ctx.enter_context(tc.tile_pool(name="nat", bufs=2))
        norm = ctx.enter_context(tc.tile_pool(name="norm", bufs=2))
        small = ctx.enter_context(tc.tile_pool(name="small", bufs=2))
        rows = ctx.enter_context(tc.tile_pool(name="rows", bufs=3))
        scr = ctx.enter_context(tc.tile_pool(name="scr", bufs=2))
        hp = ctx.enter_context(tc.tile_pool(name="hp", bufs=2))
        ps_pool = ctx.enter_context(tc.tile_pool(name="ps", bufs=2, space="PSUM"))
        st_pool = ps_pool

        # DMA queue roles: sync carries the steady-state reshape stream;
        # gpsimd carries loads/consts/outputs so they never sit behind a
        # dependency-blocked reshape (in-order queues).
        def dma_ld(out, in_):
            nc.gpsimd.dma_start(out=out, in_=in_)

        def dma_rs(out, in_):
            nc.sync.dma_start(out=out, in_=in_)

        lhsT_main_sb = singles.tile([K, P], bf16)
        dma_ld(lhsT_main_sb, lhsT_main_d[:, :])
        bias_sb = singles.tile([P, 1], f32)
        dma_ld(bias_sb, bias_d[:, :])
        lhsT_tail_sb = singles.tile([P, BINS], f32)
        dma_ld(lhsT_tail_sb, lhsT_tail_d[:, :])
        blk_sb = singles.tile([P, 2], f32)
        dma_ld(blk_sb, blk_d[:, :])
        blkT_sb = singles.tile([2, P], f32)
        dma_ld(blkT_sb, blkT_d[:, :])
        ones_sb = singles.tile([1, P], f32)
        dma_ld(ones_sb, ones_d[:, :])
        ident_sb = singles.tile([P, P], f32)
        dma_ld(ident_sb, ident_d[:, :])

        def load_and_norm(p):
            zA = nat.tile([P, F], f32, tag="zA")
            dma_ld(zA, pred_d[p, :].rearrange("(p f) -> p f", p=P))
            zB = nat.tile([P, F], f32, tag="zB")
            dma_ld(zB, targ_d[p, :].rearrange("(p f) -> p f", p=P))

            def norm_one(z, tag):
                # per-partition (min, -max) over the free dim
                mm = small.tile([P, 2], f32, tag=f"mm{tag}")
                nc.vector.tensor_reduce(out=mm[:, 0:1], in_=z, axis=AX.X, op=OP.min)
                nc.vector.tensor_reduce(
                    out=mm[:, 1:2], in_=z, axis=AX.X, op=OP.max, negate=True
                )
                # finish the cross-partition reduction via PE transpose:
                # one min-reduce of [2,128] gives (zmin, -zmax)
                t1p = st_pool.tile([2, P], f32, tag="ps")
                nc.tensor.transpose(t1p, mm, ident_sb[:, :])
                t1 = small.tile([2, P], f32, tag=f"t1{tag}")
                nc.vector.tensor_copy(t1, t1p)
                mn2 = small.tile([2, 1], f32, tag=f"mn2{tag}")
                nc.vector.tensor_reduce(out=mn2, in_=t1, axis=AX.X, op=OP.min)
                # gather (zmin, -zmax) onto partition 0
                t2p = st_pool.tile([1, 2], f32, tag="ps")
                nc.tensor.transpose(t2p, mn2, ident_sb[0:2, 0:2])
                t2 = small.tile([1, 2], f32, tag=f"t2{tag}")
                nc.vector.tensor_copy(t2, t2p)
                # sc = [-zmin, 1/(zmax - zmin + eps)] on partition 0
                sc = small.tile([1, 2], f32, tag=f"sc{tag}")
                nc.vector.tensor_scalar_mul(sc[0:1, 0:1], t2[0:1, 0:1], -1.0)
                r = small.tile([1, 1], f32, tag=f"r{tag}")
                # r = -((-zmax) + zmin) + eps = zmax - zmin + eps
                nc.vector.tensor_scalar(
                    r, t2[0:1, 1:2], t2[0:1, 0:1], -1.0, OP.add, OP.mult
                )
                nc.vector.tensor_scalar_add(r, r, EPS)
                nc.vector.reciprocal(sc[0:1, 1:2], r)
                # broadcast to all partitions with a ones-column matmul
                nbp = st_pool.tile([P, 2], f32, tag="ps")
                nc.tensor.matmul(nbp, ones_sb[:, :], sc, start=True, stop=True)
                nb = small.tile([P, 2], f32, tag=f"nb{tag}")
                nc.vector.tensor_copy(nb, nbp)
                zp = norm.tile([P, F], f32, tag=f"zp{tag}")
                nc.vector.tensor_scalar(zp, z, nb[:, 0:1], nb[:, 1:2], OP.add, OP.mult)
                zp2 = norm.tile([P, F], f32, tag=f"zp2{tag}")
                nc.vector.tensor_mul(zp2, zp, zp)
                # bf16 hi/lo splits (combined exact to ~2^-17)
                zhi = norm.tile([P, F], bf16, tag=f"zhi{tag}")
                nc.vector.tensor_copy(zhi, zp)
                zlo = norm.tile([P, F], bf16, tag=f"zlo{tag}")
                nc.vector.tensor_sub(zlo, zp, zhi)
                z2hi = norm.tile([P, F], bf16, tag=f"z2hi{tag}")
                nc.vector.tensor_copy(z2hi, zp2)
                z2lo = norm.tile([P, F], bf16, tag=f"z2lo{tag}")
                nc.vector.tensor_sub(z2lo, zp2, z2hi)
                return zhi, zlo, z2hi, z2lo

            rowsA = norm_one(zA, "A")
            rowsB = norm_one(zB, "B")
            # rhs row order must match lhsT_main rows
            return [rowsA[0], rowsA[0], rowsA[1], rowsA[2], rowsA[3],
                    rowsB[0], rowsB[0], rowsB[1], rowsB[2], rowsB[3]]

        mm_per_chunk = CHUNK // MMN  # 32
        blocks = []  # list of (start_mm, n_mm) per ACT block
        i = 0
        while i < mm_per_chunk:
            n = min(ACTB, mm_per_chunk - i)
            blocks.append((i, n))
            i += n
        pp = CHUNK // F  # natural partitions per chunk (32)

        srcs_p = [load_and_norm(p) for p in range(SPC)]
        hparts_p = []
        for p in range(SPC):
            hparts_t = hp.tile(
                [P, NCHUNK * len(blocks)], f32, tag=f"hparts{p}", name=f"hparts{p}"
            )
            hparts_p.append(hparts_t)
        # interleave the two samples' chunk pipelines so the ACT stream
        # stays dense across the whole kernel (no pair-boundary stall)
        for ch in range(NCHUNK):
            for p in range(SPC):
                srcs = srcs_p[p]
                hparts = hparts_p[p]
                rt = rows.tile([K, CHUNK], bf16, tag="rt")
                sl = slice(ch * pp, (ch + 1) * pp)
                for r, src in enumerate(srcs):
                    dma_rs(rt[r : r + 1, :], src[sl, :])
                for bi, (mm0, nmm) in enumerate(blocks):
                    ps = ps_pool.tile([P, ACTB * MMN], f32, tag="ps")
                    for k in range(nmm):
                        col = (mm0 + k) * MMN
                        nc.tensor.matmul(
                            ps[:, k * MMN : (k + 1) * MMN],
                            lhsT_main_sb[:, :],
                            rt[:, col : col + MMN],
                            start=True,
                            stop=True,
                        )
                    sc_t = scr.tile([P, ACTB * MMN], f32, tag="sc")
                    icol = ch * len(blocks) + bi
                    nc.scalar.activation(
                        out=sc_t[:, : nmm * MMN],
                        in_=ps[:, : nmm * MMN],
                        func=ACTF.Exp,
                        bias=bias_sb[:, 0:1],
                        scale=-ALPHA,
                        accum_out=hparts[:, icol : icol + 1],
                    )

        for p in range(SPC):
            hparts = hparts_p[p]
            hcol = small.tile([P, 1], f32, tag="hcol")
            nc.vector.tensor_reduce(out=hcol, in_=hparts, axis=AX.X, op=OP.add)
            heps = small.tile([P, 1], f32, tag="heps")
            nc.vector.tensor_scalar_add(heps, hcol, EPS)
            # segmented sums over the two 64-bin halves via 0/1 matmul,
            # reciprocal, then segmented broadcast via the transposed block
            s2p = st_pool.tile([2, 1], f32, tag="ps")
            nc.tensor.matmul(s2p, blk_sb[:, :], heps, start=True, stop=True)
            sinv2 = small.tile([2, 1], f32, tag="sinv2")
            nc.vector.reciprocal(sinv2, s2p)
            sbp = st_pool.tile([P, 1], f32, tag="ps")
            nc.tensor.matmul(sbp, blkT_sb[:, :], sinv2, start=True, stop=True)
            sinv = small.tile([P, 1], f32, tag="sinv")
            nc.vector.tensor_copy(sinv, sbp)
            hn = small.tile([P, 1], f32, tag="hn")
            nc.vector.tensor_mul(hn, heps, sinv)
            pst = st_pool.tile([BINS, 1], f32, tag="ps")
            nc.tensor.matmul(pst, lhsT_tail_sb[:, :], hn, start=True, stop=True)
            sq = small.tile([BINS, 1], f32, tag="sq")
            nc.scalar.square(sq, pst)
            dma_ld(out_d[p, :], sq[:, 0:1])

    nc.compile()
    return nc


def kernel(pred: np.ndarray, target: np.ndarray) -> np.ndarray:
    from concourse.bass_utils import run_bass_kernel_spmd

    if "nc" not in _CACHE:
        _CACHE["nc"] = _build_nc()
    nc = _CACHE["nc"]

    pred = np.ascontiguousarray(np.asarray(pred, np.float32).reshape(B, N))
    target = np.ascontiguousarray(np.asarray(target, np.float32).reshape(B, N))
    in_maps = [
        {
            "pred": pred[i * SPC : (i + 1) * SPC],
            "target": target[i * SPC : (i + 1) * SPC],
        }
        for i in range(NCORES)
    ]
    res = run_bass_kernel_spmd(nc, in_maps, list(range(NCORES)))
    sq = np.concatenate([r["out_sq"] for r in res.results], axis=0)  # [16, 64]
    return np.float32(np.mean(sq, dtype=np.float64))
